# revision 1
# baseline (speedup 1.0000x reference)
"""PCEN (per-channel energy normalization) Trainium2 Bass kernel.

Computation (matches the reference nn module):
    m_t = (1-S)*m_{t-1} + S*x_t  along time (last axis), m_{-1} = 0, S = 0.5
    out = (x / (EPS + m)**alpha + delta)**r - delta**r

Strategy: shard the 1024 frequency rows across 8 NeuronCores (128 rows per
core = exactly one SBUF partition dim). Per core, stream ragged time tiles
(small at the edges for fast pipeline ramp):
    DMA in -> DVE tensor_tensor_scan (EMA; since S=0.5, m_t=0.5*(m_{t-1}+x_t))
    -> ACT Ln/Exp (pow via exp(a*ln(b)); both funcs resolve to ONE activation
       table set via the _Bacc override below, avoiding ~1.3us reloads)
    -> DVE mul (u = x*p, in place on the x tile)
    -> ACT Ln/Exp -> subtract delta^r (DVE; ACT-Copy for the last tiles)
    -> DMA out.
Tiles are scan-independent: each is seeded with a 64-column halo lookback
(EMA forgets at 0.5/step => < 3e-20 truncation, exact in fp32), so there is
no serial carry chain between tiles and the scheduler can run everything as
a free pipeline. Measured ~158us on hardware per 8-core run (engine busy:
DVE ~134us = scan 67 + mul 34 + sub 16 + sync; ACT ~134us = 4 transcendental
passes; HBM floor for the 128MiB in + 128MiB out is ~94us/core).
"""

import numpy as np

S = 0.5
EPS = 1e-6

N_CORES = 8
ROWS = 1024
T_FULL = 32768
RS = ROWS // N_CORES  # 128 rows per core == SBUF partition count


def _build_and_run(x, alpha_f, r_f, delta_f, trace=False, tmpdir=None):
    import concourse.bacc as bacc
    import concourse.mybir as mybir
    import concourse.tile as tile
    from concourse.bass_utils import run_bass_kernel_spmd

    fp32 = mybir.dt.float32
    Alu = mybir.AluOpType
    Act = mybir.ActivationFunctionType

    delta_r = float(delta_f) ** float(r_f)

    class _Bacc(bacc.Bacc):
        """Bacc whose activation-table pass prefers sets covering ALL the
        activation functions this kernel uses, so interleaved Ln/Exp resolve
        to one combined table set (e.g. natural_log_exp_and_others) instead
        of thrashing between per-function sets (~1.3us per reload)."""

        def insert_act_table_loads(self):
            import bass_rust as _bass_rust
            from concourse.hw_specs import get_activation_tables

            used = {
                i.func
                for b in self.main_func.blocks
                for i in b.instructions
                if isinstance(i, mybir.InstActivation)
            }
            if not used:
                return
            tables = []
            for name, fns in get_activation_tables(self.m.arch).items():
                inter = fns & used
                # A set may serve our functions only if it contains every
                # function we use that it overlaps with... simplest safe rule:
                # if the set doesn't contain ALL used fns, strip the used fns
                # from it so the selector must pick a covering set (index is
                # preserved; ids still match act_info.json).
                if inter and not used.issubset(fns):
                    fns = fns - used
                tables.append((name, fns))
            if not any(used.issubset(fns) for _, fns in tables):
                # No single covering set exists; fall back to default policy.
                tables = list(get_activation_tables(self.m.arch).items())
            _bass_rust.insert_act_table_loads(self, tables)

    nc = _Bacc(
        "TRN2", target_bir_lowering=False, debug=False, num_devices=N_CORES
    )
    x_ap = nc.dram_tensor("x", [RS, T_FULL], fp32, kind="ExternalInput").ap()
    y_ap = nc.dram_tensor("y", [RS, T_FULL], fp32, kind="ExternalOutput").ap()

    # Ragged tiling: small tiles at the start (fast pipeline fill) and at the
    # end (short serial drain chain); big tiles in the middle.
    sizes = [512, 1024, 2048] + [4096] * 6 + [2048, 1024, 1024, 512]
    assert sum(sizes) == T_FULL
    # Each tile's scan is seeded by a HALO-column lookback instead of the
    # previous tile's carry: the EMA forgets at 0.5/step, so 64 warmup steps
    # leave < 3e-20 absolute error -- exact in fp32. This makes every tile's
    # scan independent (no serial chain, no carry tiles).
    HALO = 64

    with tile.TileContext(nc) as tc:
        with (
            tc.tile_pool(name="const", bufs=1) as cpool,
            tc.tile_pool(name="xu", bufs=5) as xpool,
            tc.tile_pool(name="m", bufs=5) as mpool,
        ):
            half = cpool.tile([RS, max(sizes) + HALO], fp32)
            # Split the memset so the first (small) scans only wait ~1us for
            # their slice of the 0.5-constant, not the whole 4160-col fill.
            nc.gpsimd.memset(half[:, : sizes[0]], 1.0 - S)
            nc.gpsimd.memset(half[:, sizes[0] :], 1.0 - S)
            eps_b = cpool.tile([RS, 1], fp32, tag="eps_b")
            nc.gpsimd.memset(eps_b[:], float(EPS))
            delta_b = cpool.tile([RS, 1], fp32, tag="delta_b")
            nc.gpsimd.memset(delta_b[:], float(delta_f))

            start = 0
            for i, size in enumerate(sizes):
                halo = HALO if i > 0 else 0
                n = size + halo  # columns in this tile incl. warmup
                xt = xpool.tile([RS, n], fp32, tag="xu")
                nc.sync.dma_start(xt[:], x_ap[:, start - halo : start + size])

                mt = mpool.tile([RS, n], fp32, tag="m")
                # m_t = (x_t + m_{t-1}) * 0.5  == EMA with S = 0.5
                nc.vector.tensor_tensor_scan(
                    mt[:],
                    xt[:],
                    half[:, :n],
                    initial=0.0,
                    op0=Alu.add,
                    op1=Alu.mult,
                )
                mv = mt[:, halo:n]
                xv = xt[:, halo:n]
                # m <- ln(m + EPS)
                nc.scalar.activation(mv, mv, Act.Ln, bias=eps_b[:])
                # m <- exp(-alpha * m) == (EPS + m)^(-alpha)
                nc.scalar.activation(mv, mv, Act.Exp, scale=-float(alpha_f))
                # x <- x * m
                nc.vector.tensor_tensor(xv, xv, mv, Alu.mult)
                # x <- ln(x + delta)
                nc.scalar.activation(xv, xv, Act.Ln, bias=delta_b[:])
                # x <- exp(r * x)
                nc.scalar.activation(xv, xv, Act.Exp, scale=float(r_f))
                # x <- x - delta^r.  For the trailing tiles DVE has drained
                # its scans and ACT has slack, so run the subtract on ACT
                # (Copy allows a float immediate bias and is in every table
                # set); earlier tiles keep it on the (otherwise busy) DVE.
                if i >= len(sizes) - 2:
                    nc.scalar.activation(
                        xv, xv, Act.Copy, bias=-delta_r, scale=1.0
                    )
                else:
                    nc.vector.tensor_scalar(xv, xv, delta_r, None, Alu.subtract)

                nc.sync.dma_start(y_ap[:, start : start + size], xv)
                start += size

    nc.compile()

    in_maps = [
        {"x": np.ascontiguousarray(x[c * RS : (c + 1) * RS])}
        for c in range(N_CORES)
    ]
    res = run_bass_kernel_spmd(
        nc, in_maps, list(range(N_CORES)), trace=trace, tmpdir=tmpdir
    )
    out = np.concatenate(
        [res.results[c]["y"] for c in range(N_CORES)], axis=0
    ).astype(np.float32)
    return out, res


def kernel(x, alpha, r, delta):
    x = np.asarray(x, dtype=np.float32)
    assert x.shape == (ROWS, T_FULL), x.shape
    out, _ = _build_and_run(x, float(alpha), float(r), float(delta))
    return out

